# revision 8
# baseline (speedup 1.0000x reference)
"""DigitCaps (CapsNet dynamic routing) kernel for 8 Trainium2 NeuronCores.

Reference math:
  u_hat[b,r,c,o] = sum_i W[r,c,o,i] * x[b,r,i]
  b_ij = 0;  3 routing iterations:
     c = softmax_r(b);  s[b,c,o] = sum_r c[r,c] u_hat[b,r,c,o];
     v = squash(s);     b += mean_b(sum_o u_hat[b,r,c,o] v[b,c,o])
  returns v[..., None]  (256, 10, 16, 1)

Strategy: ZERO-COMMUNICATION full replication.  The routing logits b_ij are
batch-shared, so the routing trajectory is identical on every core; each
core computes it for the FULL batch (B=256) for iterations 0-1 (whose only
product is the shared b_ij update), then computes the final-iteration
capsule outputs only for its OWN 32-batch slice (per-core xto input).  No
collectives, no remote DMA, no cross-core sync of any kind.

u_hat (189 MB) is never materialized: the routing coefficients are folded
into the weights so every pass is a dense matmul over K=(i,r)=9216:
    s-matmul:  s[b,(c,o)]   = sum_K  XTF[K,b] * (c-scaled Wg)[K,(c,o)]
    G-matmul:  G[K,(c,o)]   = sum_b  XN[b,K] * (v[b,(c,o)]/B)
    agreement: abar[r,c]    = sum_{i,o} Wg .* G
Rows are ordered (i, r) with r-major tiles of 128, so partition p of tile
u=(i,T) holds r = T*128+p.  x is DMA'd in both layouts (K-major XTF for
the s-matmul, b-major XN halves for the G-matmul), chunk-interleaved in
first-use order so every pass streams behind the serial DMA device.  The
o-reduction of abar is a tree of packed bf16 adds run per u-quarter so it
pipelines into the G drains, the i-reduction paired adds per quarter, the
softmax a partition_all_reduce + free-axis reduce, and the c-fold a packed
bf16 broadcast multiply (DVE-heavy split; Pool tensor ops model ~2.5x
slower per element).  PSUM accumulation chains each own a full bank (2 KB
zero region).  Matmuls run in bf16; softmax and squash in fp32.
"""
import sys
if '/opt/trn_rl_repo' not in sys.path:
    sys.path.insert(0, '/opt/trn_rl_repo')
import numpy as np
import ml_dtypes

import concourse.bass as bass
import concourse.bacc as bacc
import concourse.mybir as mybir
import concourse.tile as tile
from concourse import bass_utils
from concourse import bass_isa

BF16 = mybir.dt.bfloat16
F32 = mybir.dt.float32
ALU = mybir.AluOpType
ACT = mybir.ActivationFunctionType

B, R, C, O, I = 256, 1152, 10, 16, 8
NCORES = 8
BL = B // NCORES          # 32 own batch (final pass only)
RT = 9                    # r tiles of 128 (per i)
NT = 72                   # (i, r) tiles of 128: u = i*RT + T
CO = C * O                # 160, free order (c,o): idx = c*O + o
NITER = 3

_CACHE = {}


def _build(n_cores=NCORES, repeat=1):
    nc = bacc.Bacc("TRN2", target_bir_lowering=False, debug=False,
                   num_devices=n_cores)
    wg_d = nc.dram_tensor("wg", [128, NT * CO], BF16, kind="ExternalInput")
    xtf_d = nc.dram_tensor("xtf", [128, NT * B], BF16, kind="ExternalInput")
    xto_d = nc.dram_tensor("xto", [128, NT * BL], BF16, kind="ExternalInput")
    xn0_d = nc.dram_tensor("xn0", [128, NT * 128], BF16, kind="ExternalInput")
    xn1_d = nc.dram_tensor("xn1", [128, NT * 128], BF16, kind="ExternalInput")
    out_d = nc.dram_tensor("out", [80, 2 * BL], F32, kind="ExternalOutput")

    with tile.TileContext(nc) as tc:
        with (
            tc.tile_pool(name="big", bufs=1) as big,
            tc.tile_pool(name="small", bufs=1) as small,
            tc.tile_pool(name="sps", bufs=1, space="PSUM") as sps,
            tc.tile_pool(name="gps", bufs=2, space="PSUM") as gps,
        ):
            Wg = big.tile([128, NT * CO], BF16, tag="Wg")
            Wp = big.tile([128, NT * CO], BF16, tag="Wp")
            XTF = big.tile([128, NT * B], BF16, tag="XTF")
            XTO = big.tile([128, NT * BL], BF16, tag="XTO")
            XN0 = big.tile([128, NT * 128], BF16, tag="XN0")
            XN1 = big.tile([128, NT * 128], BF16, tag="XN1")
            Gb = big.tile([128, NT * CO], BF16, tag="Gb")
            T1 = big.tile([128, NT * C * 8], BF16, tag="T1")
            T2 = big.tile([128, NT * C * 4], BF16, tag="T2")
            T3 = big.tile([128, NT * C * 2], BF16, tag="T3")
            Q = big.tile([128, NT * C], F32, tag="Q")

            b_sb = small.tile([128, RT * C], F32, tag="b")
            expb = small.tile([128, RT * C], F32, tag="expb")
            esum = small.tile([128, RT * C], F32, tag="esum")
            c_sb = small.tile([128, RT * C], F32, tag="c")
            c16 = small.tile([128, RT * C], BF16, tag="c16")
            crep = small.tile([128, RT * C * O], BF16, tag="crep")
            zp = small.tile([128, C], F32, tag="zp")
            zr = small.tile([128, C], F32, tag="zr")
            # squash working set, full batch: [128, (h, co)] = [128, 320]
            se = small.tile([128, 2 * CO], F32, tag="se")
            ab = small.tile([128, 2 * CO], F32, tag="ab")
            sq = small.tile([128, 2 * CO], F32, tag="sq")
            rd = small.tile([128, 2 * CO], F32, tag="rd")
            num = small.tile([128, 2 * CO], F32, tag="num")
            vv = small.tile([128, 2 * CO], F32, tag="vv")
            vbf = small.tile([128, 2 * CO], BF16, tag="vbf")

            for _rep in range(repeat):
                # --- input DMAs, interleaved by first use (DMA dev serializes)
                NWC = NT // 8          # Wg chunk: 9 tiles
                NNC = NT // 4          # XN chunk: 18 tiles
                for ch in range(8):
                    nc.sync.dma_start(
                        out=Wg[:, ch * NWC * CO:(ch + 1) * NWC * CO],
                        in_=wg_d[:, ch * NWC * CO:(ch + 1) * NWC * CO])
                    nc.sync.dma_start(
                        out=XTF[:, ch * NWC * B:(ch + 1) * NWC * B],
                        in_=xtf_d[:, ch * NWC * B:(ch + 1) * NWC * B])
                for ch in range(4):
                    sl = slice(ch * NNC * 128, (ch + 1) * NNC * 128)
                    nc.sync.dma_start(out=XN0[:, sl], in_=xn0_d[:, sl])
                    nc.sync.dma_start(out=XN1[:, sl], in_=xn1_d[:, sl])
                nc.sync.dma_start(out=XTO[:, :], in_=xto_d[:, :])
                nc.vector.memset(b_sb[:, :], 0.0)

                # --- persistent PSUM tiles (one bank per accumulation chain)
                sh0 = sps.tile([128, CO], F32, tag="sh0")
                sh1 = sps.tile([128, CO], F32, tag="sh1")
                s_h = [sh0, sh1]
                warm_ps = sps.tile([128, 4], F32, tag="warm")

                def warm(src, p=128):
                    """Keep the PE p-state ramped through engine-idle windows:
                    a 4-row dummy matmul whose moving operand is the output of
                    the op that gates the next real PE work."""
                    if src.dtype != BF16:
                        src = src.bitcast(BF16)
                    nc.tensor.matmul(warm_ps[0:1, 0:4], Wg[0:p, 0:1],
                                     src[0:p, 0:4], start=True, stop=True)

                for k in range(NITER):
                    if k > 0:
                        # c = softmax over r (partitions x RT tiles)
                        nc.scalar.activation(expb[:, :], b_sb[:, :], ACT.Exp)
                        warm(expb[:, :])
                        nc.gpsimd.partition_all_reduce(
                            esum[:, :], expb[:, :], channels=128,
                            reduce_op=bass_isa.ReduceOp.add)
                        warm(esum[:, :])
                        nc.vector.tensor_reduce(
                            zp[:, :],
                            esum[:, :].rearrange("p (T c) -> p c T", c=C),
                            axis=mybir.AxisListType.X, op=ALU.add)
                        warm(zp[:, :])
                        nc.vector.reciprocal(zr[:, :], zp[:, :])
                        nc.vector.tensor_tensor(
                            c16[:, :].rearrange("p (T c) -> p T c", c=C),
                            expb[:, :].rearrange("p (T c) -> p T c", c=C),
                            zr[:, :].unsqueeze(1).broadcast_to((128, RT, C)),
                            op=ALU.mult)
                        warm(c16[:, :])
                        # crep[p,(T,c,o)] = c16[p,(T,c)] replicated over o,
                        # built in T-halves interleaved with the i-plane-0
                        # fold halves so the s-matmul's first tiles unblock
                        # one half-copy earlier
                        FI = RT * C * O  # 1440, one i-plane
                        TS = [(0, 5 * C * O), (5 * C * O, FI)]
                        for (t0, t1) in TS:
                            nc.vector.tensor_copy(
                                crep[:, t0:t1].rearrange(
                                    "p (f o) -> p f o", o=O),
                                c16[:, t0 // O:t1 // O]
                                .rearrange("p f -> p f")
                                .unsqueeze(2).broadcast_to(
                                    (128, (t1 - t0) // O, O)))
                            nc.vector.tensor_tensor(
                                Wp[:, t0:t1], Wg[:, t0:t1],
                                crep[:, t0:t1], op=ALU.mult)
                            if t0 == 0:
                                warm(Wp[:, 0:4])
                        # remaining i-planes fold full-width; i 5 on Pool
                        for ii in range(1, I):
                            eng = nc.gpsimd if ii == 5 else nc.vector
                            eng.tensor_tensor(
                                Wp[:, ii * FI:(ii + 1) * FI],
                                Wg[:, ii * FI:(ii + 1) * FI],
                                crep[:, :], op=ALU.mult)

                    mov = Wg if k == 0 else Wp

                    if k < NITER - 1:
                        # s matmul, full batch: out [b-half, co], 1 bank each
                        for u in range(NT):
                            for h in range(2):
                                nc.tensor.matmul(
                                    s_h[h][:, :],
                                    XTF[:, u * B + h * 128:u * B + h * 128 + 128],
                                    mov[:, u * CO:(u + 1) * CO],
                                    start=(u == 0), stop=(u == NT - 1))
                        P, width = 128, 2 * CO
                        sq_src = [(s_h[0][:, :], 0, CO), (s_h[1][:, :], CO, CO)]
                    else:
                        # final pass: own 32 batches only, swapped orientation
                        for u in range(NT):
                            for hh in range(2):
                                nc.tensor.matmul(
                                    s_h[hh][0:80, 0:BL],
                                    mov[:, u * CO + hh * 80:u * CO + hh * 80 + 80],
                                    XTO[:, u * BL:(u + 1) * BL],
                                    start=(u == 0), stop=(u == NT - 1))
                        P, width = 80, 2 * BL
                        sq_src = [(s_h[0][0:80, 0:BL], 0, BL),
                                  (s_h[1][0:80, 0:BL], BL, BL)]

                    # squash: v = s*|s| / (1+s^2); the two PSUM reads go
                    # to different engines so they run concurrently
                    scl = 1.0 / R if k == 0 else 1.0
                    (src0, off0, w0), (src1, off1, w1) = sq_src
                    nc.scalar.activation(se[0:P, off0:off0 + w0], src0,
                                         ACT.Copy, scale=scl)
                    if scl == 1.0:
                        nc.vector.tensor_copy(se[0:P, off1:off1 + w1], src1)
                    else:
                        nc.vector.tensor_scalar_mul(se[0:P, off1:off1 + w1],
                                                    src1, scl)
                    warm(se[:, :], p=P)
                    nc.scalar.activation(ab[0:P, 0:width], se[0:P, 0:width],
                                         ACT.Abs)
                    warm(ab[:, :], p=P)
                    nc.vector.tensor_mul(sq[0:P, 0:width], se[0:P, 0:width],
                                         se[0:P, 0:width])
                    warm(sq[:, :], p=P)
                    nc.vector.tensor_scalar_add(sq[0:P, 0:width],
                                                sq[0:P, 0:width], 1.0)
                    nc.vector.reciprocal(rd[0:P, 0:width], sq[0:P, 0:width])
                    warm(rd[:, :], p=P)
                    nc.gpsimd.tensor_tensor(num[0:P, 0:width],
                                            se[0:P, 0:width],
                                            ab[0:P, 0:width], op=ALU.mult)
                    warm(num[:, :], p=P)
                    nc.vector.tensor_mul(vv[0:P, 0:width], num[0:P, 0:width],
                                         rd[0:P, 0:width])
                    warm(vv[:, :], p=P)

                    if k == NITER - 1:
                        nc.sync.dma_start(out=out_d[:, :], in_=vv[0:80, 0:width])
                        continue

                    for hh2 in range(2):
                        nc.scalar.activation(vbf[:, hh2 * CO:(hh2 + 1) * CO],
                                             vv[:, hh2 * CO:(hh2 + 1) * CO],
                                             ACT.Copy, scale=1.0 / B)
                    warm(vbf[:, :])

                    # G matmul over b (2 chained halves per region); PSUM
                    # drains to bf16 round-robin ACT/Pool; the agreement
                    # P-mult + o-tree runs per u-quarter so only the last
                    # quarter's tree is exposed past the final G matmul.
                    # 12 big-groups of 6 K-tiles; each lands in a 2-bank PSUM
                    # tile (3 regions per bank, 480+pad layout) and drains in
                    # one strided ACT copy (GPSIMD cannot touch PSUM).
                    # Uneven chunks (in big-groups of 6 K-tiles): the early
                    # ones hide under the G matmul stream; the last is a
                    # single big-group so the exposed tree tail is short.
                    BOUNDS = [(0, 3), (3, 6), (6, 9), (9, 12)]
                    for ci, (b0, b1) in enumerate(BOUNDS):
                        for bg in range(b0, b1):
                            g_ps = gps.tile([128, 1024], F32, tag="g")
                            for j in range(6):
                                u = 6 * bg + j
                                col = (j // 3) * 512 + (j % 3) * CO
                                for h, XN in ((0, XN0), (1, XN1)):
                                    nc.tensor.matmul(
                                        g_ps[:, col:col + CO],
                                        XN[:, u * 128:(u + 1) * 128],
                                        vbf[:, h * CO:(h + 1) * CO],
                                        start=(h == 0), stop=(h == 1))
                            nc.scalar.copy(
                                Gb[:, bg * 6 * CO:(bg + 1) * 6 * CO]
                                .rearrange("p (s f) -> p s f", s=2),
                                g_ps[:, :].rearrange("p (s f) -> p s f",
                                                     s=2)[:, :, 0:480])
                        # P = Wg .* Gb for this chunk (packed bf16); one
                        # half of the non-last chunks goes to Pool.  The
                        # o-tree is column-local, so for the LAST chunk it is
                        # sub-split per 960-col block and pipelined behind
                        # each per-big-group P-mult: only the final block's
                        # chain is exposed past the last drain.
                        qb = b0 * 6 * CO
                        QW = (b1 - b0) * 6 * CO
                        last = ci == len(BOUNDS) - 1
                        nch = 3 if last else 2

                        def tree(tb, tw, wm):
                            Pv = Wp[:, tb:tb + tw].rearrange(
                                "p (f o) -> p f o", o=O)
                            T1v = T1[:, tb // 2:tb // 2 + tw // 2].rearrange(
                                "p (f o) -> p f o", o=O // 2)
                            nc.vector.tensor_tensor(
                                T1v, Pv[:, :, 0:8], Pv[:, :, 8:16],
                                op=ALU.add)
                            if wm:
                                warm(T1[:, tb // 2:tb // 2 + 4])
                            T2v = T2[:, tb // 4:tb // 4 + tw // 4].rearrange(
                                "p (f o) -> p f o", o=O // 4)
                            nc.vector.tensor_tensor(
                                T2v, T1v[:, :, 0:4], T1v[:, :, 4:8],
                                op=ALU.add)
                            if wm:
                                warm(T2[:, tb // 4:tb // 4 + 4])
                            T3v = T3[:, tb // 8:tb // 8 + tw // 8].rearrange(
                                "p (f o) -> p f o", o=O // 8)
                            nc.vector.tensor_tensor(
                                T3v, T2v[:, :, 0:2], T2v[:, :, 2:4],
                                op=ALU.add)
                            if wm:
                                warm(T3[:, tb // 8:tb // 8 + 4])
                            nc.vector.tensor_tensor(
                                Q[:, tb // 16:tb // 16 + tw // 16].rearrange(
                                    "p (f o) -> p f o", o=1),
                                T3v[:, :, 0:1], T3v[:, :, 1:2], op=ALU.add)
                            if wm:
                                warm(Q[:, tb // 16:tb // 16 + 2])

                        for hch in range(nch):
                            w = QW // nch
                            sl = slice(qb + hch * w, qb + (hch + 1) * w)
                            eng = nc.gpsimd if (hch == 0 and not last) \
                                else nc.vector
                            eng.tensor_tensor(Wp[:, sl], Wg[:, sl],
                                              Gb[:, sl], op=ALU.mult)
                            if last:
                                warm(Wp[:, sl])
                        tree(qb, QW, last)
                        # each chunk covers exactly 2 i-planes: fold the
                        # pair and accumulate into b right away so only the
                        # last chunk's chain is exposed past the G matmuls
                        qq, w2 = qb // 16, QW // 32
                        nc.vector.tensor_add(Q[:, qq:qq + w2],
                                             Q[:, qq:qq + w2],
                                             Q[:, qq + w2:qq + 2 * w2])
                        nc.vector.tensor_add(b_sb[:, :], b_sb[:, :],
                                             Q[:, qq:qq + w2])
                        if last:
                            warm(b_sb[:, :])



    nc.compile()
    return nc


def _host_inputs(x, W):
    # rows ordered (i, r), r-major tiles: tile u = i*RT + T holds
    # r = T*128 + p on partition p.  Free order (c, o).  Partition-major.
    wg = np.ascontiguousarray(
        W.transpose(3, 0, 1, 2).reshape(I, RT, 128, CO)
        .transpose(2, 0, 1, 3).reshape(128, NT * CO)).astype(ml_dtypes.bfloat16)
    xtf = np.ascontiguousarray(
        x.transpose(2, 1, 0).reshape(I, RT, 128, B)
        .transpose(2, 0, 1, 3).reshape(128, NT * B)).astype(ml_dtypes.bfloat16)
    xn = [np.ascontiguousarray(
        x[h * 128:(h + 1) * 128].transpose(0, 2, 1).reshape(128, NT * 128))
        .astype(ml_dtypes.bfloat16) for h in range(2)]
    in_maps = []
    for cidx in range(NCORES):
        xc = x[cidx * BL:(cidx + 1) * BL]          # (BL, R, I)
        xto = np.ascontiguousarray(
            xc.transpose(2, 1, 0).reshape(I, RT, 128, BL)
            .transpose(2, 0, 1, 3).reshape(128, NT * BL)).astype(
            ml_dtypes.bfloat16)
        in_maps.append({"wg": wg, "xtf": xtf, "xto": xto,
                        "xn0": xn[0], "xn1": xn[1]})
    return in_maps


def kernel(x, W):
    x = np.ascontiguousarray(np.asarray(x, dtype=np.float32))
    W = np.ascontiguousarray(np.asarray(W, dtype=np.float32))
    assert x.shape == (B, R, I) and W.shape == (R, C, O, I)
    if "nc" not in _CACHE:
        _CACHE["nc"] = _build()
    nc = _CACHE["nc"]
    in_maps = _host_inputs(x, W)
    res = bass_utils.run_bass_kernel_spmd(nc, in_maps,
                                          core_ids=list(range(NCORES)))
    # out [80, 2*BL]: out[p, hh*BL + b] = v[b_own, co = hh*80 + p]
    vs = []
    for r in res.results:
        o = r["out"].reshape(80, 2, BL).transpose(1, 0, 2).reshape(CO, BL)
        vs.append(o.T.reshape(BL, C, O))
    return np.concatenate(vs, axis=0)[..., None].astype(np.float32)


# revision 9
# speedup vs baseline: 1.0041x; 1.0041x over previous
"""DigitCaps (CapsNet dynamic routing) kernel for 8 Trainium2 NeuronCores.

Reference math:
  u_hat[b,r,c,o] = sum_i W[r,c,o,i] * x[b,r,i]
  b_ij = 0;  3 routing iterations:
     c = softmax_r(b);  s[b,c,o] = sum_r c[r,c] u_hat[b,r,c,o];
     v = squash(s);     b += mean_b(sum_o u_hat[b,r,c,o] v[b,c,o])
  returns v[..., None]  (256, 10, 16, 1)

Strategy: ZERO-COMMUNICATION full replication.  The routing logits b_ij are
batch-shared, so the routing trajectory is identical on every core; each
core computes it for the FULL batch (B=256) for iterations 0-1 (whose only
product is the shared b_ij update), then computes the final-iteration
capsule outputs only for its OWN 32-batch slice (per-core xto input).  No
collectives, no remote DMA, no cross-core sync of any kind.

u_hat (189 MB) is never materialized: the routing coefficients are folded
into the weights so every pass is a dense matmul over K=(i,r)=9216:
    s-matmul:  s[b,(c,o)]   = sum_K  XTF[K,b] * (c-scaled Wg)[K,(c,o)]
    G-matmul:  G[K,(c,o)]   = sum_b  XN[b,K] * (v[b,(c,o)]/B)
    agreement: abar[r,c]    = sum_{i,o} Wg .* G
Rows are ordered (i, r) with r-major tiles of 128, so partition p of tile
u=(i,T) holds r = T*128+p.  x is DMA'd in both layouts (K-major XTF for
the s-matmul, b-major XN halves for the G-matmul), chunk-interleaved in
first-use order so every pass streams behind the serial DMA device.  The
o-reduction of abar is a tree of packed bf16 adds run per u-quarter so it
pipelines into the G drains, the i-reduction paired adds per quarter, the
softmax a partition_all_reduce + free-axis reduce, and the c-fold a packed
bf16 broadcast multiply (DVE-heavy split; Pool tensor ops model ~2.5x
slower per element).  PSUM accumulation chains each own a full bank (2 KB
zero region).  Matmuls run in bf16; softmax and squash in fp32.
"""
import sys
if '/opt/trn_rl_repo' not in sys.path:
    sys.path.insert(0, '/opt/trn_rl_repo')
import numpy as np
import ml_dtypes

import concourse.bass as bass
import concourse.bacc as bacc
import concourse.mybir as mybir
import concourse.tile as tile
from concourse import bass_utils
from concourse import bass_isa

BF16 = mybir.dt.bfloat16
F32 = mybir.dt.float32
ALU = mybir.AluOpType
ACT = mybir.ActivationFunctionType

B, R, C, O, I = 256, 1152, 10, 16, 8
NCORES = 8
BL = B // NCORES          # 32 own batch (final pass only)
RT = 9                    # r tiles of 128 (per i)
NT = 72                   # (i, r) tiles of 128: u = i*RT + T
CO = C * O                # 160, free order (c,o): idx = c*O + o
NITER = 3

_CACHE = {}


def _build(n_cores=NCORES, repeat=1):
    nc = bacc.Bacc("TRN2", target_bir_lowering=False, debug=False,
                   num_devices=n_cores)
    wg_d = nc.dram_tensor("wg", [128, NT * CO], BF16, kind="ExternalInput")
    xtf_d = nc.dram_tensor("xtf", [128, NT * B], BF16, kind="ExternalInput")
    xto_d = nc.dram_tensor("xto", [128, NT * BL], BF16, kind="ExternalInput")
    xn0_d = nc.dram_tensor("xn0", [128, NT * 128], BF16, kind="ExternalInput")
    xn1_d = nc.dram_tensor("xn1", [128, NT * 128], BF16, kind="ExternalInput")
    out_d = nc.dram_tensor("out", [80, 2 * BL], F32, kind="ExternalOutput")

    with tile.TileContext(nc) as tc:
        with (
            tc.tile_pool(name="big", bufs=1) as big,
            tc.tile_pool(name="small", bufs=1) as small,
            tc.tile_pool(name="sps", bufs=1, space="PSUM") as sps,
            tc.tile_pool(name="gps", bufs=2, space="PSUM") as gps,
        ):
            Wg = big.tile([128, NT * CO], BF16, tag="Wg")
            Wp = big.tile([128, NT * CO], BF16, tag="Wp")
            XTF = big.tile([128, NT * B], BF16, tag="XTF")
            XTO = big.tile([128, NT * BL], BF16, tag="XTO")
            XN0 = big.tile([128, NT * 128], BF16, tag="XN0")
            XN1 = big.tile([128, NT * 128], BF16, tag="XN1")
            Gb = big.tile([128, NT * CO], BF16, tag="Gb")
            T1 = big.tile([128, NT * C * 8], BF16, tag="T1")
            T2 = big.tile([128, NT * C * 4], BF16, tag="T2")
            T3 = big.tile([128, NT * C * 2], BF16, tag="T3")
            Q = big.tile([128, NT * C], F32, tag="Q")

            b_sb = small.tile([128, RT * C], F32, tag="b")
            expb = small.tile([128, RT * C], F32, tag="expb")
            esum = small.tile([128, RT * C], F32, tag="esum")
            c_sb = small.tile([128, RT * C], F32, tag="c")
            c16 = small.tile([128, RT * C], BF16, tag="c16")
            crep = small.tile([128, RT * C * O], BF16, tag="crep")
            zp = small.tile([128, C], F32, tag="zp")
            zr = small.tile([128, C], F32, tag="zr")
            # squash working set, full batch: [128, (h, co)] = [128, 320]
            se = small.tile([128, 2 * CO], F32, tag="se")
            ab = small.tile([128, 2 * CO], F32, tag="ab")
            sq = small.tile([128, 2 * CO], F32, tag="sq")
            rd = small.tile([128, 2 * CO], F32, tag="rd")
            num = small.tile([128, 2 * CO], F32, tag="num")
            vv = small.tile([128, 2 * CO], F32, tag="vv")
            vbf = small.tile([128, 2 * CO], BF16, tag="vbf")

            for _rep in range(repeat):
                # --- input DMAs, interleaved by first use (DMA dev serializes)
                NWC = NT // 8          # Wg chunk: 9 tiles
                NNC = NT // 4          # XN chunk: 18 tiles
                for ch in range(8):
                    nc.sync.dma_start(
                        out=Wg[:, ch * NWC * CO:(ch + 1) * NWC * CO],
                        in_=wg_d[:, ch * NWC * CO:(ch + 1) * NWC * CO])
                    nc.sync.dma_start(
                        out=XTF[:, ch * NWC * B:(ch + 1) * NWC * B],
                        in_=xtf_d[:, ch * NWC * B:(ch + 1) * NWC * B])
                for ch in range(4):
                    sl = slice(ch * NNC * 128, (ch + 1) * NNC * 128)
                    nc.sync.dma_start(out=XN0[:, sl], in_=xn0_d[:, sl])
                    nc.sync.dma_start(out=XN1[:, sl], in_=xn1_d[:, sl])
                nc.sync.dma_start(out=XTO[:, :], in_=xto_d[:, :])
                nc.vector.memset(b_sb[:, :], 0.0)

                # --- persistent PSUM tiles (one bank per accumulation chain)
                sh0 = sps.tile([128, CO], F32, tag="sh0")
                sh1 = sps.tile([128, CO], F32, tag="sh1")
                s_h = [sh0, sh1]
                warm_ps = sps.tile([128, 4], F32, tag="warm")

                def warm(src, p=128):
                    """Keep the PE p-state ramped through engine-idle windows:
                    a 4-row dummy matmul whose moving operand is the output of
                    the op that gates the next real PE work."""
                    if src.dtype != BF16:
                        src = src.bitcast(BF16)
                    nc.tensor.matmul(warm_ps[0:1, 0:4], Wg[0:p, 0:1],
                                     src[0:p, 0:4], start=True, stop=True)

                for k in range(NITER):
                    if k > 0:
                        # c = softmax over r (partitions x RT tiles)
                        nc.scalar.activation(expb[:, :], b_sb[:, :], ACT.Exp)
                        warm(expb[:, :])
                        nc.gpsimd.partition_all_reduce(
                            esum[:, :], expb[:, :], channels=128,
                            reduce_op=bass_isa.ReduceOp.add)
                        warm(esum[:, :])
                        nc.vector.tensor_reduce(
                            zp[:, :],
                            esum[:, :].rearrange("p (T c) -> p c T", c=C),
                            axis=mybir.AxisListType.X, op=ALU.add)
                        warm(zp[:, :])
                        nc.vector.reciprocal(zr[:, :], zp[:, :])
                        nc.vector.tensor_tensor(
                            c16[:, :].rearrange("p (T c) -> p T c", c=C),
                            expb[:, :].rearrange("p (T c) -> p T c", c=C),
                            zr[:, :].unsqueeze(1).broadcast_to((128, RT, C)),
                            op=ALU.mult)
                        warm(c16[:, :])
                        # crep[p,(T,c,o)] = c16[p,(T,c)] replicated over o,
                        # built in T-halves interleaved with the i-plane-0
                        # fold halves so the s-matmul's first tiles unblock
                        # one half-copy earlier
                        FI = RT * C * O  # 1440, one i-plane
                        TS = [(0, 5 * C * O), (5 * C * O, FI)]
                        for (t0, t1) in TS:
                            nc.vector.tensor_copy(
                                crep[:, t0:t1].rearrange(
                                    "p (f o) -> p f o", o=O),
                                c16[:, t0 // O:t1 // O]
                                .rearrange("p f -> p f")
                                .unsqueeze(2).broadcast_to(
                                    (128, (t1 - t0) // O, O)))
                            nc.vector.tensor_tensor(
                                Wp[:, t0:t1], Wg[:, t0:t1],
                                crep[:, t0:t1], op=ALU.mult)
                            if t0 == 0:
                                warm(Wp[:, 0:4])
                        # remaining i-planes fold full-width; i 5 on Pool
                        for ii in range(1, I):
                            eng = nc.gpsimd if ii == 5 else nc.vector
                            eng.tensor_tensor(
                                Wp[:, ii * FI:(ii + 1) * FI],
                                Wg[:, ii * FI:(ii + 1) * FI],
                                crep[:, :], op=ALU.mult)

                    mov = Wg if k == 0 else Wp

                    if k < NITER - 1:
                        # s matmul, full batch: out [b-half, co], 1 bank each
                        for u in range(NT):
                            for h in range(2):
                                nc.tensor.matmul(
                                    s_h[h][:, :],
                                    XTF[:, u * B + h * 128:u * B + h * 128 + 128],
                                    mov[:, u * CO:(u + 1) * CO],
                                    start=(u == 0), stop=(u == NT - 1))
                        P, width = 128, 2 * CO
                        sq_src = [(s_h[0][:, :], 0, CO), (s_h[1][:, :], CO, CO)]
                    else:
                        # final pass: own 32 batches only, swapped orientation
                        for u in range(NT):
                            for hh in range(2):
                                nc.tensor.matmul(
                                    s_h[hh][0:80, 0:BL],
                                    mov[:, u * CO + hh * 80:u * CO + hh * 80 + 80],
                                    XTO[:, u * BL:(u + 1) * BL],
                                    start=(u == 0), stop=(u == NT - 1))
                        P, width = 80, 2 * BL
                        sq_src = [(s_h[0][0:80, 0:BL], 0, BL),
                                  (s_h[1][0:80, 0:BL], BL, BL)]

                    # squash: v = s*|s| / (1+s^2); the two PSUM reads go
                    # to different engines so they run concurrently
                    scl = 1.0 / R if k == 0 else 1.0
                    fin = k == NITER - 1   # no PE work follows the last squash
                    (src0, off0, w0), (src1, off1, w1) = sq_src
                    nc.scalar.activation(se[0:P, off0:off0 + w0], src0,
                                         ACT.Copy, scale=scl)
                    if scl == 1.0:
                        nc.vector.tensor_copy(se[0:P, off1:off1 + w1], src1)
                    else:
                        nc.vector.tensor_scalar_mul(se[0:P, off1:off1 + w1],
                                                    src1, scl)
                    if not fin:
                        warm(se[:, :], p=P)
                    nc.scalar.activation(ab[0:P, 0:width], se[0:P, 0:width],
                                         ACT.Abs)
                    if not fin:
                        warm(ab[:, :], p=P)
                    nc.vector.tensor_mul(sq[0:P, 0:width], se[0:P, 0:width],
                                         se[0:P, 0:width])
                    if not fin:
                        warm(sq[:, :], p=P)
                    nc.vector.tensor_scalar_add(sq[0:P, 0:width],
                                                sq[0:P, 0:width], 1.0)
                    nc.vector.reciprocal(rd[0:P, 0:width], sq[0:P, 0:width])
                    if not fin:
                        warm(rd[:, :], p=P)
                    nc.gpsimd.tensor_tensor(num[0:P, 0:width],
                                            se[0:P, 0:width],
                                            ab[0:P, 0:width], op=ALU.mult)
                    if not fin:
                        warm(num[:, :], p=P)
                    nc.vector.tensor_mul(vv[0:P, 0:width], num[0:P, 0:width],
                                         rd[0:P, 0:width])
                    if not fin:
                        warm(vv[:, :], p=P)

                    if k == NITER - 1:
                        nc.sync.dma_start(out=out_d[:, :], in_=vv[0:80, 0:width])
                        continue

                    for hh2 in range(2):
                        nc.scalar.activation(vbf[:, hh2 * CO:(hh2 + 1) * CO],
                                             vv[:, hh2 * CO:(hh2 + 1) * CO],
                                             ACT.Copy, scale=1.0 / B)
                    warm(vbf[:, :])

                    # G matmul over b (2 chained halves per region); PSUM
                    # drains to bf16 round-robin ACT/Pool; the agreement
                    # P-mult + o-tree runs per u-quarter so only the last
                    # quarter's tree is exposed past the final G matmul.
                    # 12 big-groups of 6 K-tiles; each lands in a 2-bank PSUM
                    # tile (3 regions per bank, 480+pad layout) and drains in
                    # one strided ACT copy (GPSIMD cannot touch PSUM).
                    # Uneven chunks (in big-groups of 6 K-tiles): the early
                    # ones hide under the G matmul stream; the last is a
                    # single big-group so the exposed tree tail is short.
                    BOUNDS = [(0, 3), (3, 6), (6, 9), (9, 12)]
                    for ci, (b0, b1) in enumerate(BOUNDS):
                        for bg in range(b0, b1):
                            g_ps = gps.tile([128, 1024], F32, tag="g")
                            for j in range(6):
                                u = 6 * bg + j
                                col = (j // 3) * 512 + (j % 3) * CO
                                for h, XN in ((0, XN0), (1, XN1)):
                                    nc.tensor.matmul(
                                        g_ps[:, col:col + CO],
                                        XN[:, u * 128:(u + 1) * 128],
                                        vbf[:, h * CO:(h + 1) * CO],
                                        start=(h == 0), stop=(h == 1))
                            nc.scalar.copy(
                                Gb[:, bg * 6 * CO:(bg + 1) * 6 * CO]
                                .rearrange("p (s f) -> p s f", s=2),
                                g_ps[:, :].rearrange("p (s f) -> p s f",
                                                     s=2)[:, :, 0:480])
                        # P = Wg .* Gb for this chunk (packed bf16); one
                        # half of the non-last chunks goes to Pool.  The
                        # o-tree is column-local, so for the LAST chunk it is
                        # sub-split per 960-col block and pipelined behind
                        # each per-big-group P-mult: only the final block's
                        # chain is exposed past the last drain.
                        qb = b0 * 6 * CO
                        QW = (b1 - b0) * 6 * CO
                        last = ci == len(BOUNDS) - 1
                        nch = 3 if last else 2

                        def tree(tb, tw, wm):
                            Pv = Wp[:, tb:tb + tw].rearrange(
                                "p (f o) -> p f o", o=O)
                            T1v = T1[:, tb // 2:tb // 2 + tw // 2].rearrange(
                                "p (f o) -> p f o", o=O // 2)
                            nc.vector.tensor_tensor(
                                T1v, Pv[:, :, 0:8], Pv[:, :, 8:16],
                                op=ALU.add)
                            if wm:
                                warm(T1[:, tb // 2:tb // 2 + 4])
                            T2v = T2[:, tb // 4:tb // 4 + tw // 4].rearrange(
                                "p (f o) -> p f o", o=O // 4)
                            nc.vector.tensor_tensor(
                                T2v, T1v[:, :, 0:4], T1v[:, :, 4:8],
                                op=ALU.add)
                            if wm:
                                warm(T2[:, tb // 4:tb // 4 + 4])
                            T3v = T3[:, tb // 8:tb // 8 + tw // 8].rearrange(
                                "p (f o) -> p f o", o=O // 8)
                            nc.vector.tensor_tensor(
                                T3v, T2v[:, :, 0:2], T2v[:, :, 2:4],
                                op=ALU.add)
                            if wm:
                                warm(T3[:, tb // 8:tb // 8 + 4])
                            nc.vector.tensor_tensor(
                                Q[:, tb // 16:tb // 16 + tw // 16].rearrange(
                                    "p (f o) -> p f o", o=1),
                                T3v[:, :, 0:1], T3v[:, :, 1:2], op=ALU.add)
                            if wm:
                                warm(Q[:, tb // 16:tb // 16 + 2])

                        for hch in range(nch):
                            w = QW // nch
                            sl = slice(qb + hch * w, qb + (hch + 1) * w)
                            eng = nc.gpsimd if (hch == 0 and not last) \
                                else nc.vector
                            eng.tensor_tensor(Wp[:, sl], Wg[:, sl],
                                              Gb[:, sl], op=ALU.mult)
                            if last:
                                warm(Wp[:, sl])
                        tree(qb, QW, last)
                        # each chunk covers exactly 2 i-planes: fold the
                        # pair and accumulate into b right away so only the
                        # last chunk's chain is exposed past the G matmuls
                        qq, w2 = qb // 16, QW // 32
                        nc.vector.tensor_add(Q[:, qq:qq + w2],
                                             Q[:, qq:qq + w2],
                                             Q[:, qq + w2:qq + 2 * w2])
                        nc.vector.tensor_add(b_sb[:, :], b_sb[:, :],
                                             Q[:, qq:qq + w2])
                        if last:
                            warm(b_sb[:, :])



    nc.compile()
    return nc


def _host_inputs(x, W):
    # rows ordered (i, r), r-major tiles: tile u = i*RT + T holds
    # r = T*128 + p on partition p.  Free order (c, o).  Partition-major.
    wg = np.ascontiguousarray(
        W.transpose(3, 0, 1, 2).reshape(I, RT, 128, CO)
        .transpose(2, 0, 1, 3).reshape(128, NT * CO)).astype(ml_dtypes.bfloat16)
    xtf = np.ascontiguousarray(
        x.transpose(2, 1, 0).reshape(I, RT, 128, B)
        .transpose(2, 0, 1, 3).reshape(128, NT * B)).astype(ml_dtypes.bfloat16)
    xn = [np.ascontiguousarray(
        x[h * 128:(h + 1) * 128].transpose(0, 2, 1).reshape(128, NT * 128))
        .astype(ml_dtypes.bfloat16) for h in range(2)]
    in_maps = []
    for cidx in range(NCORES):
        xc = x[cidx * BL:(cidx + 1) * BL]          # (BL, R, I)
        xto = np.ascontiguousarray(
            xc.transpose(2, 1, 0).reshape(I, RT, 128, BL)
            .transpose(2, 0, 1, 3).reshape(128, NT * BL)).astype(
            ml_dtypes.bfloat16)
        in_maps.append({"wg": wg, "xtf": xtf, "xto": xto,
                        "xn0": xn[0], "xn1": xn[1]})
    return in_maps


def kernel(x, W):
    x = np.ascontiguousarray(np.asarray(x, dtype=np.float32))
    W = np.ascontiguousarray(np.asarray(W, dtype=np.float32))
    assert x.shape == (B, R, I) and W.shape == (R, C, O, I)
    if "nc" not in _CACHE:
        _CACHE["nc"] = _build()
    nc = _CACHE["nc"]
    in_maps = _host_inputs(x, W)
    res = bass_utils.run_bass_kernel_spmd(nc, in_maps,
                                          core_ids=list(range(NCORES)))
    # out [80, 2*BL]: out[p, hh*BL + b] = v[b_own, co = hh*80 + p]
    vs = []
    for r in res.results:
        o = r["out"].reshape(80, 2, BL).transpose(1, 0, 2).reshape(CO, BL)
        vs.append(o.T.reshape(BL, C, O))
    return np.concatenate(vs, axis=0)[..., None].astype(np.float32)
